# revision 21
# baseline (speedup 1.0000x reference)
"""Bass/Trainium2 kernel for nn_BakaAttention: 8-way data-parallel over batch.

Per core (one batch element):
  q = rope(x@wq, off=1024); k = rope(concat(past_k, x@wk), off=0); v = concat(past_v, x@wv)
  out = softmax(mask(q k^T / 16)) v @ wo

All matmul operands are bf16 (1 cycle/row on PE, half the DMA/SBUF bytes);
PSUM accumulation stays f32. The host pre-arranges every input into its exact
SBUF tile layout (x transposed, past_k feature-major, weights tiled) so each
tensor lands in one contiguous DMA, and pre-permutes q/k feature order per head
to [even feats | odd feats], which turns interleaved-pair rope into
partition-aligned elementwise ops (no on-chip transposes or rotation matmuls).
Scores are computed transposed [s, t] so PV consumes probs directly as the
moving operand; everything stays SBUF-resident (no DRAM round trips).
"""

import numpy as np

B, T, P, H, DH, DIN, DOUT = 8, 1024, 1024, 4, 256, 1024, 1152
S = P + T  # 2048 keys
THETA = 10000.0
NCORES = 8


def _bf16():
    import ml_dtypes
    return ml_dtypes.bfloat16


def _host_constants():
    bf16 = _bf16()
    m = np.arange(128, dtype=np.float64)
    inv = 1.0 / (THETA ** (m / 128.0))                      # [128]
    pos = np.arange(S, dtype=np.float64)                    # [2048]
    ang = np.outer(inv, pos)                                # [128, 2048]
    consts = {
        "cos": np.cos(ang).astype(bf16),
        "sin": np.sin(ang).astype(bf16),
    }
    # triangular keep-mask for the 128-wide diagonal chunk: keep sl <= tl
    sl = np.arange(128)[:, None]
    tl = np.arange(128)[None, :]
    consts["masks"] = np.ascontiguousarray((sl <= tl).astype(bf16))
    consts["ones"] = np.ones((128, 1), bf16)
    consts["onesr"] = np.ones((1, 128), np.float32)
    return consts


# column permutation putting each head's even features first, then odd
def _qk_perm():
    idx = np.arange(DIN).reshape(H, DH // 2, 2)
    return np.concatenate([idx[..., 0], idx[..., 1]], axis=1).reshape(-1)


def build_kernel():
    import concourse.bass as bass
    import concourse.mybir as mybir
    from concourse import bacc
    from concourse.tile import TileContext

    f32 = mybir.dt.float32
    f32r = mybir.dt.float32r
    bf = mybir.dt.bfloat16
    AF = mybir.ActivationFunctionType
    OP = mybir.AluOpType

    nc = bacc.Bacc(None, target_bir_lowering=False)

    # host-prearranged layouts: leading dims match SBUF [partition, slab, col]
    xT_d = nc.dram_tensor("xT", [128, 8, T], bf, kind="ExternalInput")
    pkT_d = nc.dram_tensor("pkT", [128, 8, P], bf, kind="ExternalInput")
    pv_d = nc.dram_tensor("pv", [128, 8, DIN], bf, kind="ExternalInput")
    wq_d = nc.dram_tensor("wq", [8, 128, 4, 256], bf, kind="ExternalInput")
    wk_d = nc.dram_tensor("wk", [4, 128, 8, 256], bf, kind="ExternalInput")
    wv_d = nc.dram_tensor("wv", [128, 8, DIN], bf, kind="ExternalInput")
    wo_d = nc.dram_tensor("wo", [128, 8, DOUT], bf, kind="ExternalInput")
    cos_d = nc.dram_tensor("cos", [128, S], bf, kind="ExternalInput")
    sin_d = nc.dram_tensor("sin", [128, S], bf, kind="ExternalInput")
    masks_d = nc.dram_tensor("masks", [128, 128], bf, kind="ExternalInput")
    ones_d = nc.dram_tensor("ones", [128, 1], bf, kind="ExternalInput")
    onesr_d = nc.dram_tensor("onesr", [1, 128], f32r, kind="ExternalInput")
    out_d = nc.dram_tensor("out", [T, DOUT], bf, kind="ExternalOutput")

    from contextlib import ExitStack
    stack = ExitStack()
    with TileContext(nc) as tc, stack:
        cstp = stack.enter_context(tc.tile_pool(name="consts", bufs=1))
        masks = cstp.tile([128, 128], bf, name="masks", tag="masks")
        ones_sb = cstp.tile([128, 1], bf, name="ones_sb", tag="ones_sb")
        onesr_sb = cstp.tile([1, 128], f32r, name="onesr_sb", tag="onesr_sb")
        cos_sb = cstp.tile([128, S], bf, name="cos_sb", tag="cos_sb")
        sin_sb = cstp.tile([128, S], bf, name="sin_sb", tag="sin_sb")

        resid = stack.enter_context(tc.tile_pool(name="resid", bufs=1))
        kT = [resid.tile([128, S], bf, name=f"kT{i}", tag=f"kT{i}") for i in range(8)]
        qT = [resid.tile([128, T], bf, name=f"qT{i}", tag=f"qT{i}") for i in range(8)]
        vbig = resid.tile([128, 16, DIN], bf, name="vbig", tag="vbig")
        yT = [resid.tile([128, T], bf, name=f"yT{i}", tag=f"yT{i}") for i in range(8)]

        # ---------------- Phase A: projections + rope ----------------
        with tc.tile_pool(name="paxT", bufs=1) as xtp, \
             tc.tile_pool(name="pawv", bufs=1) as wvp, \
             tc.tile_pool(name="papk", bufs=1) as pkp, \
             tc.tile_pool(name="pawt", bufs=2) as wtp, \
             tc.tile_pool(name="patmp", bufs=2) as tmp, \
             tc.tile_pool(name="paraw", bufs=2) as rawp, \
             tc.tile_pool(name="paps", bufs=4, space="PSUM") as psp:
            # startup-critical DMAs first: first w tile + x slabs, split
            # across both DGE queues (sync + scalar) for issue overlap.
            wth_all = [wtp.tile([128, 8, 256], bf, name="wload", tag="wload",
                                bufs=3) for i in range(8)]
            nc.scalar.dma_start(out=wth_all[0][:, 0:4, :], in_=wq_d[0])
            nc.sync.dma_start(out=wth_all[0][:, 4:8, :], in_=wq_d[1])
            x_sb = [xtp.tile([128, T], bf, name=f"x{i}", tag=f"x{i}")
                    for i in range(8)]
            nc.sync.dma_start(out=x_sb[0][:], in_=xT_d[:, 0, :])
            nc.sync.dma_start(out=x_sb[1][:], in_=xT_d[:, 1, :])
            nc.scalar.dma_start(out=x_sb[2][:], in_=xT_d[:, 2, :])
            nc.sync.dma_start(out=x_sb[3][:], in_=xT_d[:, 3, :])
            nc.scalar.dma_start(out=x_sb[4][:], in_=xT_d[:, 4, :])
            nc.sync.dma_start(out=x_sb[5][:], in_=xT_d[:, 5, :])
            nc.scalar.dma_start(out=x_sb[6][:], in_=xT_d[:, 6, :])
            nc.sync.dma_start(out=wth_all[1][:, 0:4, :], in_=wq_d[2])
            nc.scalar.dma_start(out=wth_all[1][:, 4:8, :], in_=wq_d[3])
            nc.scalar.dma_start(out=x_sb[7][:], in_=xT_d[:, 7, :])
            nc.sync.dma_start(out=cos_sb[:], in_=cos_d[:])
            nc.scalar.dma_start(out=sin_sb[:], in_=sin_d[:])
            pkbig = pkp.tile([128, 8, P], bf, name="pkbig", tag="pkbig")
            nc.scalar.dma_start(out=pkbig[:], in_=pkT_d[:])
            nc.sync.dma_start(out=masks[:], in_=masks_d[:])
            nc.sync.dma_start(out=ones_sb[:], in_=ones_d[:])
            nc.sync.dma_start(out=onesr_sb[:], in_=onesr_d[:])
            for i in (2, 3):
                eng = (nc.sync, nc.scalar)[i % 2]
                eng.dma_start(out=wth_all[i][:, 0:4, :], in_=wq_d[2 * i])
                eng.dma_start(out=wth_all[i][:, 4:8, :], in_=wq_d[2 * i + 1])
            for i in range(4, 8):
                eng = (nc.sync, nc.scalar)[i % 2]
                eng.dma_start(out=wth_all[i][:], in_=wk_d[i % 4])

            # past_k rope: combine directly from the DMA'd f-major tiles into
            # kT[:, 0:P]; vector-only (gpsimd contends with DVE for SBUF).
            for h in range(4):
                c, s = cos_sb[:, 0:P], sin_sb[:, 0:P]
                pe, po = pkbig[:, 2 * h, :], pkbig[:, 2 * h + 1, :]
                t1 = tmp.tile([128, 1024], bf, name="t1", tag="t1")
                t2 = tmp.tile([128, 1024], bf, name="t2", tag="t2")
                nc.vector.tensor_tensor(t1[:], pe, c, op=OP.mult)
                nc.vector.tensor_tensor(t2[:], po, s, op=OP.mult)
                nc.vector.tensor_tensor(kT[2 * h][:, 0:P], t1[:], t2[:],
                                        op=OP.subtract)
                t3 = tmp.tile([128, 1024], bf, name="t1", tag="t1")
                t4 = tmp.tile([128, 1024], bf, name="t2", tag="t2")
                nc.vector.tensor_tensor(t3[:], po, c, op=OP.mult)
                nc.vector.tensor_tensor(t4[:], pe, s, op=OP.mult)
                nc.vector.tensor_tensor(kT[2 * h + 1][:, 0:P], t3[:], t4[:],
                                        op=OP.add)

            # q/k projections per head; rope applied on the PSUM outputs.
            # Even f-tile (2h) and odd (2h+1) rows share the same cos/sin rows.
            for wi, (dst, off) in enumerate(((qT, 0), (kT, P))):
                for h in range(4):
                    wth = wth_all[4 * wi + h]
                    psl = [psp.tile([128, 512], f32, name=f"pj{i}", tag=f"pj{i}",
                                    bufs=2) for i in range(4)]
                    for kt in range(8):
                        for f2 in range(2):
                            for th in range(2):
                                nc.tensor.matmul(
                                    psl[2 * f2 + th][:],
                                    wth[:, kt, 128 * f2:128 * (f2 + 1)],
                                    x_sb[kt][:, 512 * th:512 * (th + 1)],
                                    start=(kt == 0), stop=(kt == 7))
                    c = cos_sb[:, P:P + T]
                    s = sin_sb[:, P:P + T]
                    e_sb = rawp.tile([128, 1024], bf, name="e_sb", tag="e_sb")
                    o_sb = rawp.tile([128, 1024], bf, name="o_sb", tag="o_sb")
                    for th in range(2):
                        hh = slice(512 * th, 512 * (th + 1))
                        nc.scalar.copy(e_sb[:, hh], psl[th][:])
                        nc.scalar.copy(o_sb[:, hh], psl[2 + th][:])
                    de = dst[2 * h][:, off:off + T]
                    do = dst[2 * h + 1][:, off:off + T]
                    t1 = tmp.tile([128, 1024], bf, name="t1", tag="t1")
                    t2 = tmp.tile([128, 1024], bf, name="t2", tag="t2")
                    nc.vector.tensor_tensor(t1[:], e_sb[:], c, op=OP.mult)
                    nc.vector.tensor_tensor(t2[:], o_sb[:], s, op=OP.mult)
                    nc.vector.tensor_tensor(de, t1[:], t2[:], op=OP.subtract)
                    t3 = tmp.tile([128, 1024], bf, name="t1", tag="t1")
                    t4 = tmp.tile([128, 1024], bf, name="t2", tag="t2")
                    nc.vector.tensor_tensor(t3[:], o_sb[:], c, op=OP.mult)
                    nc.vector.tensor_tensor(t4[:], e_sb[:], s, op=OP.mult)
                    nc.vector.tensor_tensor(do, t3[:], t4[:], op=OP.add)

            # past_v straight in (natural [s, h*f] layout)
            nc.sync.dma_start(out=vbig[:, 0:8, :], in_=pv_d[:])

            # v projection, natural layout [s, f]
            wvbig = wvp.tile([128, 8, DIN], bf, name="wvbig", tag="wvbig")
            nc.sync.dma_start(out=wvbig[:], in_=wv_d[:])
            for stg in range(4):
                psl = [psp.tile([128, 512], f32, name=f"pv{i}", tag=f"pj{i}",
                                bufs=2) for i in range(4)]
                for kt in range(8):
                    for s2 in range(2):
                        st = 2 * stg + s2
                        for fh in range(2):
                            nc.tensor.matmul(
                                psl[2 * s2 + fh][:],
                                x_sb[kt][:, 128 * st:128 * (st + 1)],
                                wvbig[:, kt, 512 * fh:512 * (fh + 1)],
                                start=(kt == 0), stop=(kt == 7))
                for s2 in range(2):
                    st = 2 * stg + s2
                    for fh in range(2):
                        nc.scalar.copy(vbig[:, 8 + st, 512 * fh:512 * (fh + 1)],
                                       psl[2 * s2 + fh][:])

        # ---------------- Phase B+C: attention + o-projection ----------------
        with tc.tile_pool(name="pbwo", bufs=1) as wop, \
             tc.tile_pool(name="probs", bufs=5) as prp, \
             tc.tile_pool(name="pbsm", bufs=2) as smp, \
             tc.tile_pool(name="pbo", bufs=2) as osp, \
             tc.tile_pool(name="pbsc", bufs=3, space="PSUM") as scps, \
             tc.tile_pool(name="pby", bufs=1, space="PSUM") as yps:
            wobig = wop.tile([128, 8, DOUT], bf, name="wobig", tag="wobig")
            nc.sync.dma_start(out=wobig[:], in_=wo_d[:])
            for TH in (1, 0):
                for h in range(4):
                    jmax = 12 + 4 * TH
                    ytp_ps = [yps.tile([128, 512], f32, name=f"ytp{i}",
                                       tag=f"ytp{i}", bufs=2) for i in range(2)]
                    sm_ps = yps.tile([1, 512], f32, name="smps", tag="smps",
                                     bufs=1)

                    # scores + exp + mask for block j (sw-pipelined one ahead
                    # of PV so the PE never waits on the scalar-engine exp)
                    def probs(j):
                        ci = j - (8 + 4 * TH)
                        c0 = 128 * ci if ci > 0 else 0  # first unmasked column
                        sc = scps.tile([128, 512], f32, name="sc", tag="sc")
                        for fk in range(2):
                            nc.tensor.matmul(
                                sc[:, c0:512],
                                kT[2 * h + fk][:, 128 * j:128 * (j + 1)],
                                qT[2 * h + fk][:, 512 * TH + c0:512 * (TH + 1)],
                                start=(fk == 0), stop=(fk == 1))
                        pj = prp.tile([128, 512], bf, name="pj", tag="pj")
                        nc.scalar.activation(pj[:, c0:512], sc[:, c0:512], AF.Exp,
                                             scale=float(DH ** -0.5))
                        if ci >= 0:
                            # triangular mask on the 128-wide diagonal chunk
                            nc.vector.tensor_tensor(
                                pj[:, c0:c0 + 128], pj[:, c0:c0 + 128],
                                masks[:], op=OP.mult)
                        return pj, c0

                    pjq = [probs(0), probs(1)]
                    for j in range(jmax):
                        if j + 2 < jmax:
                            pjq.append(probs(j + 2))
                        pj, c0 = pjq[j]
                        for fb in range(2):
                            nc.tensor.matmul(
                                ytp_ps[fb][:, c0:512],
                                vbig[:, j, 256 * h + 128 * fb:256 * h + 128 * (fb + 1)],
                                pj[:, c0:512],
                                start=(j == 0), stop=(j == jmax - 1))
                        nc.tensor.matmul(
                            sm_ps[:, c0:512], ones_sb[:], pj[:, c0:512],
                            start=(j == 0), stop=(j == jmax - 1))
                    # normalize off the PE queue: reciprocal of the sums row,
                    # broadcast across partitions on gpsimd, scale on vector
                    rrow = smp.tile([1, 512], f32, name="rrow", tag="rrow")
                    nc.vector.reciprocal_approx_fast(out=rrow[:], in_=sm_ps[:])
                    rbc = smp.tile([128, 512], f32, name="rbc", tag="rbc")
                    nc.gpsimd.partition_broadcast(rbc[:], rrow[:])
                    for fb in range(2):
                        nc.vector.tensor_tensor(
                            yT[2 * h + fb][:, 512 * TH:512 * (TH + 1)],
                            ytp_ps[fb][:],
                            rbc[:],
                            op=OP.mult)
                # o-projection for this TH's four t-tiles (PSUM shared with
                # the sc tag so attention of the next TH keeps its banks)
                for tt in range(4 * TH, 4 * TH + 4):
                    ot = osp.tile([128, DOUT], bf, name="osb", tag="osb")
                    for ds in range(3):
                        op_ps = scps.tile([128, 512], f32, name="sc", tag="sc")
                        for fk in range(8):
                            nc.tensor.matmul(
                                op_ps[:, 0:384],
                                yT[fk][:, 128 * tt:128 * (tt + 1)],
                                wobig[:, fk, 384 * ds:384 * (ds + 1)],
                                start=(fk == 0), stop=(fk == 7))
                        nc.scalar.copy(ot[:, 384 * ds:384 * (ds + 1)],
                                       op_ps[:, 0:384])
                    nc.sync.dma_start(out=out_d[128 * tt:128 * (tt + 1), :],
                                      in_=ot[:])

    nc.finalize()
    return nc


_NC_CACHE = {}


def _prep_inputs(x, past_k, past_v, wq, wk, wv, wo):
    bf16 = _bf16()
    consts = _host_constants()
    perm = _qk_perm()

    def wtile(w):  # [1024, 1024] -> [4 head, 128 part, 8 kt, 256 col]
        return np.ascontiguousarray(
            w.reshape(8, 128, 4, 256).transpose(2, 1, 0, 3)).astype(bf16)

    def ptile(a):  # [1024, C] -> [128 part, 8 slab, C]
        c = a.shape[-1]
        return np.ascontiguousarray(
            a.reshape(8, 128, c).transpose(1, 0, 2)).astype(bf16)

    wq_p = np.ascontiguousarray(
        wtile(wq[:, perm]).reshape(4, 128, 2, 4, 256).transpose(0, 2, 1, 3, 4)
    ).reshape(8, 128, 4, 256)
    wk_p = wtile(wk[:, perm])
    wv_b = ptile(wv)
    wo_b = ptile(wo)
    in_maps = []
    for b in range(NCORES):
        pk = past_k[b]  # [P, H, DH]
        pkp = np.concatenate([pk[:, :, 0::2], pk[:, :, 1::2]], axis=2)  # [P,H,256]
        pkT = np.ascontiguousarray(pkp.transpose(1, 2, 0).reshape(8, 128, P)
                                   .transpose(1, 0, 2)).astype(bf16)
        m = {
            "xT": ptile(np.ascontiguousarray(x[b].T)),
            "pkT": pkT,
            "pv": ptile(past_v[b].reshape(P, DIN)),
            "wq": wq_p, "wk": wk_p, "wv": wv_b, "wo": wo_b,
            "cos": consts["cos"], "sin": consts["sin"],
            "masks": consts["masks"], "ones": consts["ones"],
            "onesr": consts["onesr"],
        }
        in_maps.append(m)
    return in_maps


def run(x, past_k, past_v, wq, wk, wv, wo, trace=False):
    from concourse.bass_utils import run_bass_kernel_spmd

    if "nc" not in _NC_CACHE:
        _NC_CACHE["nc"] = build_kernel()
    nc = _NC_CACHE["nc"]
    in_maps = _prep_inputs(x, past_k, past_v, wq, wk, wv, wo)
    res = run_bass_kernel_spmd(nc, in_maps, list(range(NCORES)), trace=trace)
    out = np.stack([res.results[b]["out"].astype(np.float32) for b in range(NCORES)], axis=0)
    return out, res


def kernel(x, past_k, past_v, wq, wk, wv, wo):
    out, _ = run(x, past_k, past_v, wq, wk, wv, wo)
    return out
